# revision 11
# baseline (speedup 1.0000x reference)
"""Trainium2 Bass kernel for masked multi-head attention (B=2, S=2048, H=16, D=64).

Sharding: 8 cores = (2 batches) x (4 groups of 4 heads). Each core computes
qkv for its 4 heads + flash-style attention fully on-chip.

Host-side prep per core:
  - xT    [1024, 2048] fp16 = x[b].T (kept tokens permuted first)
  - w     [128, 6, 8, 128] fp16 = W_qkv columns for this head group, laid out
          piece-contiguous per partition so every DMA moves 2KB lines.
          Piece order: k0, q0, v0, v1, k1, q1 (128 cols each; q|k|v pairs
          cover the group's 256 output dims).
  - bias  [128, n_kv/128] = 0 for real keys, -1e9 for padding (exp -> 0)
Device returns outT [260, 2048] f32 = 4 heads x (64 out rows + 1 softmax-sum
row), host divides and transposes into the final [2, 2048, 1024].

Schedule: QKV projection for the first scores block runs first; the attention
stream over (head-pair, q-chunk, kv-block) is software-pipelined with the
exp->PV distance = 2 so the PE never waits on the Activation engine. The
remaining QKV work is queued as single-matmul filler pieces drained at a
paced rate inside the attention stream to keep the PE saturated while ACT
does the exps. PV accumulators are DMA'd to DRAM directly from PSUM.
"""

import sys

sys.path.insert(0, "/opt/trn_rl_repo")

import numpy as np

import concourse.bass as bass  # noqa: F401
import concourse.tile as tile
from concourse import bacc, mybir
from concourse.bass_utils import run_bass_kernel_spmd

B, S, DIM = 2, 2048, 1024
HEAD, HEAD_DIM = 16, 64
NEG = np.float32(-1e9)
NFI = DIM // 128  # 8 contraction tiles
F32 = mybir.dt.float32
F32R = mybir.dt.float32r
F16 = mybir.dt.float16
BF16 = mybir.dt.bfloat16

N_DUMMY = 8  # PE clock warm-up matmuls at the DMA-gated head
PULL_START_T = 4  # first attention slot allowed to drain filler pieces
DEPTH = 2  # scores->pv software-pipeline distance (in kv-block slots)

# w piece order: k0, q0, v0, v1, k1, q1 (each 128 of the group's 768 cols)
W_PIECES = [(256, 384), (0, 128), (512, 640), (640, 768), (384, 512), (128, 256)]
PIECE_K = [0, 4]  # piece index holding kT cols for pair p
PIECE_Q = [1, 5]
PIECE_V = [2, 3]

_CACHE = {}


def _chunks(total, maxc=512):
    n = -(-total // maxc)
    base = -(-total // (n * 128)) * 128
    out = []
    off = 0
    while off < total:
        w = min(base, total - off)
        out.append((off, w))
        off += w
    return out


def _emit_body(nc, tc, pools, dram, n_kv):
    NKT = n_kv // 128
    big, ps, ps_sc, ps2, ptp, osp = pools
    xT_d, w_d, bias_d, outT_d = dram
    qchunks = _chunks(S)
    kchunks = _chunks(n_kv)
    assert all(wd == 512 for _, wd in qchunks)
    Exp = mybir.ActivationFunctionType.Exp

    # preload the exp table while DMAs run
    warm = big.tile([128, 1], F32, tag="warm", name="warm")
    nc.gpsimd.memset(warm[:], 1.0)
    nc.scalar.activation(warm[:], warm[:], Exp)

    # dummy matmuls on a memset tile: keep PE busy through the DMA-gated
    # head so HAM reaches full clock before the real work arrives
    dummy = big.tile([128, 512], mybir.dt.bfloat16, tag="dummy", name="dummy")
    nc.gpsimd.memset(dummy[:], 1.0)
    dps = ps.tile([128, 512], F32, tag="wide", name="dps")
    for _ in range(N_DUMMY):
        nc.tensor.matmul(
            dps[:, 0:512], dummy[:, 0:128], dummy[:], start=True, stop=True
        )

    w_s = big.tile([128, 6, NFI, 128], F16, tag="w", name="w_s")
    w_ap = w_d.ap()
    bias_s = big.tile([128, NKT], F32, tag="bias", name="bias_s")
    xT_ap = xT_d.ap().rearrange("(a p) t -> p a t", p=128)
    xts = big.tile([128, NFI, S], F16, tag="xts", name="xts")

    # DMA order = first-needed-first; chunk 0 stripped per-fi so the first
    # accumulations pipeline with arrival
    nc.sync.dma_start(w_s[:, 0], w_ap[:, 0])  # k0
    for fi in range(NFI):
        nc.sync.dma_start(xts[:, fi, 0:512], xT_ap[:, fi, 0:512])
    nc.sync.dma_start(bias_s[:], bias_d.ap())
    nc.sync.dma_start(w_s[:, 1], w_ap[:, 1])  # q0
    for fi in range(NFI):
        nc.sync.dma_start(xts[:, fi, 512:1024], xT_ap[:, fi, 512:1024])
    nc.sync.dma_start(w_s[:, 2], w_ap[:, 2])  # v0
    nc.sync.dma_start(w_s[:, 3], w_ap[:, 3])  # v1
    nc.sync.dma_start(w_s[:, 4], w_ap[:, 4])  # k1
    nc.sync.dma_start(w_s[:, 5], w_ap[:, 5])  # q1
    for off, wd in qchunks[2:]:
        nc.sync.dma_start(xts[:, :, off : off + wd], xT_ap[:, :, off : off + wd])
    xkv = xts  # kept keys are the first n_kv (host-permuted) columns

    qT = [big.tile([128, S], F16, tag=f"qT{p}", name=f"qT{p}") for p in range(2)]
    kT = [
        big.tile([128, n_kv], F16, tag=f"kT{p}", name=f"kT{p}") for p in range(2)
    ]
    va = big.tile([128, NKT, 4, 65], BF16, tag="va", name="va")
    ones = big.tile([128, 4, 1], F32, tag="ones", name="ones")
    nc.gpsimd.memset(ones[:], 1.0)
    for jt in range(NKT):
        nc.vector.tensor_copy(va[:, jt, :, 64:65], ones[:])

    def emit_k_chunk(p, off, wd):
        assert wd == 512
        acc = ps.tile([128, 512], F32, tag="wide", name="acc_k")
        for fi in range(NFI):
            nc.tensor.matmul(
                acc[:, 0:wd],
                w_s[:, PIECE_K[p], fi, :],
                xkv[:, fi, off : off + wd],
                start=(fi == 0),
                stop=(fi == NFI - 1),
            )
        nc.vector.tensor_copy(kT[p][:, off : off + wd], acc[:, :wd])

    def emit_v_range(j0, j1):
        for jt in range(j0, j1):
            acc = ps.tile([128, 512], F32, tag="wide", name="acc_v")
            for half in range(2):
                for fi in range(NFI):
                    nc.tensor.matmul(
                        acc[:, 128 * half : 128 * half + 128],
                        xkv[:, fi, jt * 128 : (jt + 1) * 128],
                        w_s[:, PIECE_V[half], fi, :],
                        start=(fi == 0),
                        stop=(fi == NFI - 1),
                    )
                nc.vector.tensor_copy(
                    va[:, jt, 2 * half : 2 * half + 2, 0:64],
                    acc[:, 128 * half : 128 * half + 128].rearrange(
                        "p (h d) -> p h d", h=2
                    ),
                )

    def emit_q(p, ci):
        off, wd = qchunks[ci]
        acc = ps.tile([128, 512], F32, tag="wide", name="acc_q")
        for fi in range(NFI):
            nc.tensor.matmul(
                acc[:, :wd],
                w_s[:, PIECE_Q[p], fi, :],
                xts[:, fi, off : off + wd],
                start=(fi == 0),
                stop=(fi == NFI - 1),
            )
        nc.vector.tensor_copy(qT[p][:, off : off + wd], acc[:, :wd])

    # emission ordered by DMA arrival: work gated on x chunk i comes before
    # anything gated on chunk i+1; v tiles and later k chunks fold into
    # block 0's slot loop right before their first consumer
    emit_k_chunk(0, *kchunks[0])
    emit_q(0, 0)
    emit_q(0, 1)

    inner0 = {}
    for off, wd in kchunks[1:]:
        inner0.setdefault(off // 128, []).append(
            lambda off=off, wd=wd: emit_k_chunk(0, off, wd)
        )
    for jt in range(NKT):
        inner0.setdefault(min(jt + 1, NKT - 1), []).append(
            lambda jt=jt: emit_v_range(jt, jt + 1)
        )

    # fine-grained filler pieces: (needed_by_block, callable); one matmul each
    queue = []

    def q_pieces(p, ci, needed_by):
        off, wd = qchunks[ci]
        cell = []

        def mk(fi):
            def f():
                if not cell:
                    cell.append(
                        ps.tile([128, 512], F32, tag="wide", name="acc_qf")
                    )
                nc.tensor.matmul(
                    cell[0][:, :wd],
                    w_s[:, PIECE_Q[p], fi, :],
                    xts[:, fi, off : off + wd],
                    start=(fi == 0),
                    stop=(fi == NFI - 1),
                )

            return f

        for fi in range(NFI):
            queue.append((needed_by, mk(fi)))
        queue.append(
            (
                needed_by,
                lambda: nc.vector.tensor_copy(
                    qT[p][:, off : off + wd], cell[0][:, :wd]
                ),
            )
        )

    def k_pieces(p, off, wd, needed_by):
        cell = []

        def mk(sub, sw, fi):
            def f():
                if not cell:
                    cell.append(
                        ps.tile([128, 512], F32, tag="wide", name="acc_kf")
                    )
                nc.tensor.matmul(
                    cell[0][:, 0:sw],
                    w_s[:, PIECE_K[p], fi, :],
                    xkv[:, fi, off + sub : off + sub + sw],
                    start=(fi == 0),
                    stop=(fi == NFI - 1),
                )

            return f

        for sub in range(0, wd, 512):
            sw = min(512, wd - sub)
            for fi in range(NFI):
                queue.append((needed_by, mk(sub, sw, fi)))
        queue.append(
            (
                needed_by,
                lambda: nc.vector.tensor_copy(
                    kT[p][:, off : off + wd], cell[0][:, :wd]
                ),
            )
        )

    q_pieces(0, 2, needed_by=2)
    q_pieces(0, 3, needed_by=2)
    for off, wd in kchunks:
        k_pieces(1, off, wd, needed_by=4)
    q_pieces(1, 0, needed_by=4)
    q_pieces(1, 1, needed_by=4)
    q_pieces(1, 2, needed_by=6)
    q_pieces(1, 3, needed_by=6)

    # ---- attention: (pair, head, q-half) blocks; each slot loads each
    # stationary operand once and streams 1024 q columns through it ----
    blocks = [(p, i, qh) for p in range(2) for qh in range(2) for i in range(2)]
    T = len(blocks) * NKT

    def drain_required(bi):
        while queue and queue[0][0] <= bi:
            queue.pop(0)[1]()

    def pull(n):
        while n > 0 and queue:
            queue.pop(0)[1]()
            n -= 1

    def scores_exp(bi, j):
        p, i, qh = blocks[bi]
        lo = 64 * i
        sc = ps_sc.tile([128, 1024], F32, tag="sc", name="sc")
        for c in range(2):
            coff = 1024 * qh + 512 * c
            nc.tensor.matmul(
                sc[:, 512 * c : 512 * c + 512],
                kT[p][lo : lo + 64, j * 128 : (j + 1) * 128],
                qT[p][lo : lo + 64, coff : coff + 512],
                start=True,
                stop=True,
            )
        pt = ptp.tile([128, 1024], BF16, tag="pt", name="pt")
        nc.scalar.activation(pt[:], sc[:], Exp, bias=bias_s[:, j : j + 1])
        return pt

    pv_cell = [None]

    def pv_mm(bi, j, pt):
        p, i, qh = blocks[bi]
        if j == 0:
            pv_cell[0] = ps2.tile([65, 1024], F32, tag="pv", name="pv")
        pv = pv_cell[0]
        for c in range(2):
            nc.tensor.matmul(
                pv[:, 512 * c : 512 * c + 512],
                va[:, j, 2 * p + i, :],
                pt[:, 512 * c : 512 * c + 512],
                start=(j == 0),
                stop=(j == NKT - 1),
            )
            if j == NKT - 1:
                coff = 1024 * qh + 512 * c
                o = osp.tile([65, 512], F32, tag="o", name="o")
                nc.vector.tensor_copy(o[:], pv[:, 512 * c : 512 * c + 512])
                lh = 2 * p + i
                nc.sync.dma_start(
                    outT_d.ap()[65 * lh : 65 * lh + 65, coff : coff + 512],
                    o[:],
                )

    # paced filler drain: meet needed_by deadlines (~2/slot up to midpoint),
    # then stretch the remainder across the rest of the stream so the PE
    # stays fed while ACT catches up on exps
    pend = []
    carry = 0.0
    for t in range(T):
        bi, j = divmod(t, NKT)
        if j == 0:
            drain_required(bi)
        if bi == 0:
            for f in inner0.get(j, []):
                f()
        pt = scores_exp(bi, j)
        pend.append((bi, j, pt))
        if t >= PULL_START_T and queue:
            if t < T // 2:
                pull(2)
            else:
                carry += len(queue) / max(1, T - 1 - t)
                n = int(carry)
                carry -= n
                pull(min(n, 3))
        if len(pend) > DEPTH:
            pv_mm(*pend.pop(0))
    while pend:
        pv_mm(*pend.pop(0))
    drain_required(len(blocks))


def _build(n_kv: int, reps: int = 1):
    """Build the per-core Bass graph. Same graph runs SPMD on all 8 cores."""
    nc = bacc.Bacc("TRN2", target_bir_lowering=False, debug=False)

    NKT = n_kv // 128
    xT_d = nc.dram_tensor("xT", [DIM, S], F16, kind="ExternalInput")
    w_d = nc.dram_tensor("w", [128, 6, NFI, 128], F16, kind="ExternalInput")
    bias_d = nc.dram_tensor("bias", [128, NKT], F32, kind="ExternalInput")
    outT_d = nc.dram_tensor("outT", [260, S], F32, kind="ExternalOutput")
    dram = (xT_d, w_d, bias_d, outT_d)

    with tile.TileContext(nc) as tc:
        with (
            tc.tile_pool(name="big", bufs=1) as big,
            tc.tile_pool(name="ps", bufs=2, space="PSUM") as ps,
            tc.tile_pool(name="ps_sc", bufs=2, space="PSUM") as ps_sc,
            tc.tile_pool(name="ps2", bufs=1, space="PSUM") as ps2,
            tc.tile_pool(name="ptp", bufs=6) as ptp,
            tc.tile_pool(name="osp", bufs=4) as osp,
        ):
            pools = (big, ps, ps_sc, ps2, ptp, osp)
            for rep in range(reps):
                if rep:
                    tc.strict_bb_all_engine_barrier()
                _emit_body(nc, tc, pools, dram, n_kv)

    nc.compile()
    return nc


def _get_graph(n_kv: int, reps: int = 1):
    key = (n_kv, reps)
    if key not in _CACHE:
        _CACHE[key] = _build(n_kv, reps)
    return _CACHE[key]


def prepare(x, W_qkv, mask, reps: int = 1):
    """Host-side prep: returns (nc, in_maps, perms)."""
    x = np.asarray(x, dtype=np.float32)
    W_qkv = np.asarray(W_qkv, dtype=np.float32)
    mask = np.asarray(mask)

    keep = [np.nonzero(mask[b] != 0)[0] for b in range(B)]
    n_keep = max(len(k) for k in keep)
    n_kv = min(S, max(128, -(-n_keep // 128) * 128))

    # permute tokens: kept (unmasked) first, rest after; k/v use first n_kv
    perms, xT, biases = [], [], []
    for b in range(B):
        unkept = np.nonzero(mask[b] == 0)[0]
        perm = np.concatenate([keep[b], unkept])
        perms.append(perm)
        xT.append(np.ascontiguousarray(x[b][perm].T.astype(np.float16)))
        bv = np.full(n_kv, NEG, np.float32)
        bv[: len(keep[b])] = 0.0
        biases.append(np.ascontiguousarray(bv.reshape(-1, 128).T))

    wg = []
    for g in range(4):
        base = np.empty((128, 6, NFI, 128), np.float16)
        for pi, (c0, c1) in enumerate(W_PIECES):
            cols = np.empty((DIM, c1 - c0), np.float32)
            qkv_kind = [1, 0, 2, 2, 1, 0][pi]  # k0,q0,v0,v1,k1,q1 -> q/k/v base
            src0 = qkv_kind * DIM + 256 * g + (c0 % 256)
            cols[:] = W_qkv[:, src0 : src0 + (c1 - c0)]
            # [DIM, 128] -> [128 partitions, NFI, 128]
            base[:, pi] = (
                cols.reshape(NFI, 128, 128).transpose(1, 0, 2).astype(np.float16)
            )
        wg.append(base)

    in_maps = []
    for c in range(8):
        b, g = c // 4, c % 4
        in_maps.append({"xT": xT[b], "w": wg[g], "bias": biases[b]})

    nc = _get_graph(n_kv, reps)
    return nc, in_maps, perms


def assemble(results, perms):
    out = np.empty((B, S, DIM), np.float32)
    for c in range(8):
        b, g = c // 4, c % 4
        outT = results[c]["outT"]
        for i in range(4):
            h = 4 * g + i
            rows = outT[65 * i : 65 * i + 64]
            sums = outT[65 * i + 64]
            out[b, perms[b], 64 * h : 64 * (h + 1)] = (rows / sums).T
    return out


def run(x, W_qkv, mask, trace=False, tmpdir=None):
    nc, in_maps, perms = prepare(x, W_qkv, mask)
    res = run_bass_kernel_spmd(
        nc, in_maps, core_ids=list(range(8)), trace=trace, tmpdir=tmpdir
    )
    return assemble(res.results, perms), res


def kernel(x, W_qkv, mask):
    out, _ = run(x, W_qkv, mask)
    return out


# revision 13
# speedup vs baseline: 1.0059x; 1.0059x over previous
"""Trainium2 Bass kernel for masked multi-head attention (B=2, S=2048, H=16, D=64).

Sharding: 8 cores = (2 batches) x (4 groups of 4 heads). Each core computes
qkv for its 4 heads + flash-style attention fully on-chip.

Host-side prep per core:
  - xT    [1024, 2048] fp16 = x[b].T (kept tokens permuted first)
  - w     [128, 6, 8, 128] fp16 = W_qkv columns for this head group, laid out
          piece-contiguous per partition so every DMA moves 2KB lines.
          Piece order: k0, q0, v0, v1, k1, q1 (128 cols each; q|k|v pairs
          cover the group's 256 output dims).
  - bias  [128, n_kv/128] = 0 for real keys, -1e9 for padding (exp -> 0)
Device returns outT [260, 2048] f32 = 4 heads x (64 out rows + 1 softmax-sum
row), host divides and transposes into the final [2, 2048, 1024].

Schedule: QKV projection for the first scores block runs first; the attention
stream over (head-pair, q-chunk, kv-block) is software-pipelined with the
exp->PV distance = 2 so the PE never waits on the Activation engine. The
remaining QKV work is queued as single-matmul filler pieces drained at a
paced rate inside the attention stream to keep the PE saturated while ACT
does the exps. Matmul operands are fp16 (QKV, scores) / bf16 (PV, whose pt
operand needs bf16 range), which stream 3-8x faster through the PE than
fp32r on hardware.
"""

import sys

sys.path.insert(0, "/opt/trn_rl_repo")

import numpy as np

import concourse.bass as bass  # noqa: F401
import concourse.tile as tile
from concourse import bacc, mybir
from concourse.bass_utils import run_bass_kernel_spmd

B, S, DIM = 2, 2048, 1024
HEAD, HEAD_DIM = 16, 64
NEG = np.float32(-1e9)
NFI = DIM // 128  # 8 contraction tiles
F32 = mybir.dt.float32
F32R = mybir.dt.float32r
F16 = mybir.dt.float16
BF16 = mybir.dt.bfloat16

N_DUMMY = 12  # PE clock warm-up matmuls at the DMA-gated head
PULL_START_T = 4  # first attention slot allowed to drain filler pieces
DEPTH = 3  # scores->pv software-pipeline distance (in kv-block slots)

# w piece order: k0, q0, v0, v1, k1, q1 (each 128 of the group's 768 cols)
W_PIECES = [(256, 384), (0, 128), (512, 640), (640, 768), (384, 512), (128, 256)]
PIECE_K = [0, 4]  # piece index holding kT cols for pair p
PIECE_Q = [1, 5]
PIECE_V = [2, 3]

_CACHE = {}


def _chunks(total, maxc=512):
    n = -(-total // maxc)
    base = -(-total // (n * 128)) * 128
    out = []
    off = 0
    while off < total:
        w = min(base, total - off)
        out.append((off, w))
        off += w
    return out


def _emit_body(nc, tc, pools, dram, n_kv):
    NKT = n_kv // 128
    big, ps, ps2, ptp, osp = pools
    xT_d, w_d, bias_d, outT_d = dram
    qchunks = _chunks(S)
    kchunks = _chunks(n_kv)
    assert all(wd == 512 for _, wd in qchunks)
    Exp = mybir.ActivationFunctionType.Exp

    # preload the exp table while DMAs run
    warm = big.tile([128, 1], F32, tag="warm", name="warm")
    nc.gpsimd.memset(warm[:], 1.0)
    nc.scalar.activation(warm[:], warm[:], Exp)

    # dummy matmuls on a memset tile: keep PE busy through the DMA-gated
    # head so HAM reaches full clock before the real work arrives
    dummy = big.tile([128, 512], BF16, tag="dummy", name="dummy")
    nc.gpsimd.memset(dummy[:], 1.0)
    dps = ps.tile([128, 1024], F32, tag="wide", name="dps")
    for _ in range(N_DUMMY):
        nc.tensor.matmul(
            dps[:, 0:512], dummy[:, 0:128], dummy[:], start=True, stop=True
        )

    w_s = big.tile([128, 6, NFI, 128], F16, tag="w", name="w_s")
    w_ap = w_d.ap()
    bias_s = big.tile([128, NKT], F32, tag="bias", name="bias_s")
    xT_ap = xT_d.ap().rearrange("(a p) t -> p a t", p=128)
    xts = big.tile([128, NFI, S], F16, tag="xts", name="xts")

    # DMA order = first-needed-first; chunk 0 stripped per-fi so the first
    # accumulations pipeline with arrival
    nc.sync.dma_start(w_s[:, 0], w_ap[:, 0])  # k0
    for fi in range(NFI):
        nc.sync.dma_start(xts[:, fi, 0:512], xT_ap[:, fi, 0:512])
    nc.sync.dma_start(bias_s[:], bias_d.ap())
    nc.sync.dma_start(w_s[:, 1], w_ap[:, 1])  # q0
    nc.sync.dma_start(w_s[:, 2], w_ap[:, 2])  # v0
    nc.sync.dma_start(w_s[:, 3], w_ap[:, 3])  # v1
    nc.sync.dma_start(xts[:, :, 512:1024], xT_ap[:, :, 512:1024])
    nc.sync.dma_start(w_s[:, 4], w_ap[:, 4])  # k1
    nc.sync.dma_start(w_s[:, 5], w_ap[:, 5])  # q1
    for off, wd in qchunks[2:]:
        nc.sync.dma_start(xts[:, :, off : off + wd], xT_ap[:, :, off : off + wd])
    xkv = xts  # kept keys are the first n_kv (host-permuted) columns

    qT = [big.tile([128, S], F16, tag=f"qT{p}", name=f"qT{p}") for p in range(2)]
    kT = [
        big.tile([128, n_kv], F16, tag=f"kT{p}", name=f"kT{p}") for p in range(2)
    ]
    va = big.tile([128, NKT, 4, 65], BF16, tag="va", name="va")
    ones = big.tile([128, 4, 1], F32, tag="ones", name="ones")
    nc.gpsimd.memset(ones[:], 1.0)
    for jt in range(NKT):
        nc.vector.tensor_copy(va[:, jt, :, 64:65], ones[:])

    def emit_k_chunk(p, off, wd):
        acc = ps.tile([128, 1024], F32, tag="wide", name="acc_k")
        for sub in range(0, wd, 512):
            sw = min(512, wd - sub)
            for fi in range(NFI):
                nc.tensor.matmul(
                    acc[:, sub : sub + sw],
                    w_s[:, PIECE_K[p], fi, :],
                    xkv[:, fi, off + sub : off + sub + sw],
                    start=(fi == 0),
                    stop=(fi == NFI - 1),
                )
        nc.vector.tensor_copy(kT[p][:, off : off + wd], acc[:, :wd])

    def emit_v_range(j0, j1):
        for jt in range(j0, j1):
            acc = ps.tile([128, 1024], F32, tag="wide", name="acc_v")
            for half in range(2):
                for fi in range(NFI):
                    nc.tensor.matmul(
                        acc[:, 128 * half : 128 * half + 128],
                        xkv[:, fi, jt * 128 : (jt + 1) * 128],
                        w_s[:, PIECE_V[half], fi, :],
                        start=(fi == 0),
                        stop=(fi == NFI - 1),
                    )
                nc.vector.tensor_copy(
                    va[:, jt, 2 * half : 2 * half + 2, 0:64],
                    acc[:, 128 * half : 128 * half + 128].rearrange(
                        "p (h d) -> p h d", h=2
                    ),
                )

    def emit_q(p, ci):
        off, wd = qchunks[ci]
        acc = ps.tile([128, 1024], F32, tag="wide", name="acc_q")
        for fi in range(NFI):
            nc.tensor.matmul(
                acc[:, :wd],
                w_s[:, PIECE_Q[p], fi, :],
                xts[:, fi, off : off + wd],
                start=(fi == 0),
                stop=(fi == NFI - 1),
            )
        nc.vector.tensor_copy(qT[p][:, off : off + wd], acc[:, :wd])

    # emission ordered by DMA arrival: work gated on x chunk i comes before
    # anything gated on chunk i+1; v tiles and later k chunks fold into
    # block 0's slot loop right before their first consumer
    emit_k_chunk(0, *kchunks[0])
    emit_q(0, 0)

    inner0 = {}
    for off, wd in kchunks[1:]:
        inner0.setdefault(off // 128, []).append(
            lambda off=off, wd=wd: emit_k_chunk(0, off, wd)
        )
    for jt in range(NKT):
        inner0.setdefault(min(jt + 1, NKT - 1), []).append(
            lambda jt=jt: emit_v_range(jt, jt + 1)
        )

    # fine-grained filler pieces: (needed_by_block, callable); one matmul each
    queue = []

    def q_pieces(p, ci, needed_by):
        off, wd = qchunks[ci]
        cell = []

        def mk(fi):
            def f():
                if not cell:
                    cell.append(
                        ps.tile([128, 1024], F32, tag="wide", name="acc_qf")
                    )
                nc.tensor.matmul(
                    cell[0][:, :wd],
                    w_s[:, PIECE_Q[p], fi, :],
                    xts[:, fi, off : off + wd],
                    start=(fi == 0),
                    stop=(fi == NFI - 1),
                )

            return f

        for fi in range(NFI):
            queue.append((needed_by, mk(fi)))
        queue.append(
            (
                needed_by,
                lambda: nc.vector.tensor_copy(
                    qT[p][:, off : off + wd], cell[0][:, :wd]
                ),
            )
        )

    def k_pieces(p, off, wd, needed_by):
        cell = []

        def mk(sub, sw, fi):
            def f():
                if not cell:
                    cell.append(
                        ps.tile([128, 1024], F32, tag="wide", name="acc_kf")
                    )
                nc.tensor.matmul(
                    cell[0][:, sub : sub + sw],
                    w_s[:, PIECE_K[p], fi, :],
                    xkv[:, fi, off + sub : off + sub + sw],
                    start=(fi == 0),
                    stop=(fi == NFI - 1),
                )

            return f

        for sub in range(0, wd, 512):
            sw = min(512, wd - sub)
            for fi in range(NFI):
                queue.append((needed_by, mk(sub, sw, fi)))
        queue.append(
            (
                needed_by,
                lambda: nc.vector.tensor_copy(
                    kT[p][:, off : off + wd], cell[0][:, :wd]
                ),
            )
        )

    nq = len(qchunks)
    q_pieces(0, 1, needed_by=1)
    q_pieces(0, 2, needed_by=2)
    q_pieces(0, 3, needed_by=3)
    for off, wd in kchunks:
        k_pieces(1, off, wd, needed_by=nq)
    for ci in range(nq):
        q_pieces(1, ci, needed_by=nq + ci)

    # ---- attention: linearized (pair, q-chunk, kv-block) stream ----
    blocks = [(p, ci) for p in range(2) for ci in range(len(qchunks))]
    T = len(blocks) * NKT

    def drain_required(bi):
        while queue and queue[0][0] <= bi:
            queue.pop(0)[1]()

    def pull(n):
        while n > 0 and queue:
            queue.pop(0)[1]()
            n -= 1

    def scores_exp(bi, j):
        p, ci = blocks[bi]
        coff, cw = qchunks[ci]
        sc = ps.tile([128, 1024], F32, tag="wide", name="sc")
        for i in range(2):
            lo = 64 * i
            nc.tensor.matmul(
                sc[:, 512 * i : 512 * i + cw],
                kT[p][lo : lo + 64, j * 128 : (j + 1) * 128],
                qT[p][lo : lo + 64, coff : coff + cw],
                start=True,
                stop=True,
            )
        pt = ptp.tile([128, 1024], BF16, tag="pt", name="pt")
        nc.scalar.activation(pt[:], sc[:], Exp, bias=bias_s[:, j : j + 1])
        return pt

    pv_cell = [None]

    def pv_mm(bi, j, pt):
        p, ci = blocks[bi]
        coff, cw = qchunks[ci]
        if j == 0:
            pv_cell[0] = ps2.tile([65, 1024], F32, tag="pv", name="pv")
        pv = pv_cell[0]
        for i in range(2):
            nc.tensor.matmul(
                pv[:, 512 * i : 512 * i + cw],
                va[:, j, 2 * p + i, :],
                pt[:, 512 * i : 512 * i + cw],
                start=(j == 0),
                stop=(j == NKT - 1),
            )
            if j == NKT - 1:
                o = osp.tile([65, 512], F32, tag="o", name="o")
                nc.vector.tensor_copy(o[:, :cw], pv[:, 512 * i : 512 * i + cw])
                lh = 2 * p + i
                nc.sync.dma_start(
                    outT_d.ap()[65 * lh : 65 * lh + 65, coff : coff + cw],
                    o[:, :cw],
                )

    # paced filler drain: meet needed_by deadlines (~2/slot up to midpoint),
    # then stretch the remainder across the rest of the stream so the PE
    # stays fed while ACT catches up on exps
    pend = []
    carry = 0.0
    for t in range(T):
        bi, j = divmod(t, NKT)
        if j == 0:
            drain_required(bi)
        if bi == 0:
            for f in inner0.get(j, []):
                f()
        pt = scores_exp(bi, j)
        pend.append((bi, j, pt))
        if t >= PULL_START_T and queue:
            if t < T // 2:
                pull(2)
            else:
                carry += len(queue) / max(1, T - 1 - t)
                n = int(carry)
                carry -= n
                pull(min(n, 3))
        if len(pend) > DEPTH:
            pv_mm(*pend.pop(0))
    while pend:
        pv_mm(*pend.pop(0))
    drain_required(len(blocks))


def _build(n_kv: int, reps: int = 1):
    """Build the per-core Bass graph. Same graph runs SPMD on all 8 cores."""
    nc = bacc.Bacc("TRN2", target_bir_lowering=False, debug=False)

    NKT = n_kv // 128
    xT_d = nc.dram_tensor("xT", [DIM, S], F16, kind="ExternalInput")
    w_d = nc.dram_tensor("w", [128, 6, NFI, 128], F16, kind="ExternalInput")
    bias_d = nc.dram_tensor("bias", [128, NKT], F32, kind="ExternalInput")
    outT_d = nc.dram_tensor("outT", [260, S], F32, kind="ExternalOutput")
    dram = (xT_d, w_d, bias_d, outT_d)

    with tile.TileContext(nc) as tc:
        with (
            tc.tile_pool(name="big", bufs=1) as big,
            tc.tile_pool(name="ps", bufs=3, space="PSUM") as ps,
            tc.tile_pool(name="ps2", bufs=1, space="PSUM") as ps2,
            tc.tile_pool(name="ptp", bufs=6) as ptp,
            tc.tile_pool(name="osp", bufs=4) as osp,
        ):
            pools = (big, ps, ps2, ptp, osp)
            for rep in range(reps):
                if rep:
                    tc.strict_bb_all_engine_barrier()
                _emit_body(nc, tc, pools, dram, n_kv)

    nc.compile()
    return nc


def _get_graph(n_kv: int, reps: int = 1):
    key = (n_kv, reps)
    if key not in _CACHE:
        _CACHE[key] = _build(n_kv, reps)
    return _CACHE[key]


def prepare(x, W_qkv, mask, reps: int = 1):
    """Host-side prep: returns (nc, in_maps, perms)."""
    x = np.asarray(x, dtype=np.float32)
    W_qkv = np.asarray(W_qkv, dtype=np.float32)
    mask = np.asarray(mask)

    keep = [np.nonzero(mask[b] != 0)[0] for b in range(B)]
    n_keep = max(len(k) for k in keep)
    n_kv = min(S, max(128, -(-n_keep // 128) * 128))

    # permute tokens: kept (unmasked) first, rest after; k/v use first n_kv
    perms, xT, biases = [], [], []
    for b in range(B):
        unkept = np.nonzero(mask[b] == 0)[0]
        perm = np.concatenate([keep[b], unkept])
        perms.append(perm)
        xT.append(np.ascontiguousarray(x[b][perm].T.astype(np.float16)))
        bv = np.full(n_kv, NEG, np.float32)
        bv[: len(keep[b])] = 0.0
        biases.append(np.ascontiguousarray(bv.reshape(-1, 128).T))

    wg = []
    for g in range(4):
        base = np.empty((128, 6, NFI, 128), np.float16)
        for pi, (c0, c1) in enumerate(W_PIECES):
            cols = np.empty((DIM, c1 - c0), np.float32)
            qkv_kind = [1, 0, 2, 2, 1, 0][pi]  # k0,q0,v0,v1,k1,q1 -> q/k/v base
            src0 = qkv_kind * DIM + 256 * g + (c0 % 256)
            cols[:] = W_qkv[:, src0 : src0 + (c1 - c0)]
            # [DIM, 128] -> [128 partitions, NFI, 128]
            base[:, pi] = (
                cols.reshape(NFI, 128, 128).transpose(1, 0, 2).astype(np.float16)
            )
        wg.append(base)

    in_maps = []
    for c in range(8):
        b, g = c // 4, c % 4
        in_maps.append({"xT": xT[b], "w": wg[g], "bias": biases[b]})

    nc = _get_graph(n_kv, reps)
    return nc, in_maps, perms


def assemble(results, perms):
    out = np.empty((B, S, DIM), np.float32)
    for c in range(8):
        b, g = c // 4, c % 4
        outT = results[c]["outT"]
        for i in range(4):
            h = 4 * g + i
            rows = outT[65 * i : 65 * i + 64]
            sums = outT[65 * i + 64]
            out[b, perms[b], 64 * h : 64 * (h + 1)] = (rows / sums).T
    return out


def run(x, W_qkv, mask, trace=False, tmpdir=None):
    nc, in_maps, perms = prepare(x, W_qkv, mask)
    res = run_bass_kernel_spmd(
        nc, in_maps, core_ids=list(range(8)), trace=trace, tmpdir=tmpdir
    )
    return assemble(res.results, perms), res


def kernel(x, W_qkv, mask):
    out, _ = run(x, W_qkv, mask)
    return out


# revision 14
# speedup vs baseline: 1.1675x; 1.1606x over previous
"""Trainium2 Bass kernel for masked multi-head attention (B=2, S=2048, H=16, D=64).

Sharding: 8 cores = (2 batches) x (4 groups of 4 heads). Each core computes
qkv for its 4 heads + flash-style attention fully on-chip.

Host-side prep per core:
  - xT    [1024, 2048] fp16 = x[b].T (kept tokens permuted first)
  - w     [128, 6, 8, 128] fp16 = W_qkv columns for this head group, laid out
          piece-contiguous per partition so every DMA moves 2KB lines.
          Piece order: k0, q0, v0, v1, k1, q1 (128 cols each; q|k|v pairs
          cover the group's 256 output dims).
  - bias  [128, n_kv/128] = 0 for real keys, -1e9 for padding (exp -> 0)
Device returns outT [260, 2048] f32 = 4 heads x (64 out rows + 1 softmax-sum
row), host divides and transposes into the final [2, 2048, 1024].

Schedule: QKV projection for the first scores block runs first; the attention
stream over (head-pair, q-chunk, kv-block) is software-pipelined with the
exp->PV distance = 2 so the PE never waits on the Activation engine. The
remaining QKV work is queued as single-matmul filler pieces drained at a
paced rate inside the attention stream to keep the PE saturated while ACT
does the exps. Matmul operands are fp16 (QKV, scores) / bf16 (PV, whose pt
operand needs bf16 range), which stream 3-8x faster through the PE than
fp32r on hardware.
"""

import sys

sys.path.insert(0, "/opt/trn_rl_repo")

import numpy as np

import concourse.bass as bass  # noqa: F401
import concourse.tile as tile
from concourse import bacc, mybir
from concourse.bass_utils import run_bass_kernel_spmd

B, S, DIM = 2, 2048, 1024
HEAD, HEAD_DIM = 16, 64
NEG = np.float32(-1e9)
NFI = DIM // 128  # 8 contraction tiles
F32 = mybir.dt.float32
F32R = mybir.dt.float32r
F16 = mybir.dt.float16
BF16 = mybir.dt.bfloat16

N_DUMMY = 8  # PE clock warm-up matmuls at the DMA-gated head
PULL_START_T = 4  # first attention slot allowed to drain filler pieces
DEPTH = 2  # scores->pv software-pipeline distance (in kv-block slots)

# w piece order: k0, q0, v0, v1, k1, q1 (each 128 of the group's 768 cols)
W_PIECES = [(256, 384), (0, 128), (512, 640), (640, 768), (384, 512), (128, 256)]
PIECE_K = [0, 4]  # piece index holding kT cols for pair p
PIECE_Q = [1, 5]
PIECE_V = [2, 3]

_CACHE = {}


def _chunks(total, maxc=512):
    n = -(-total // maxc)
    base = -(-total // (n * 128)) * 128
    out = []
    off = 0
    while off < total:
        w = min(base, total - off)
        out.append((off, w))
        off += w
    return out


def _emit_body(nc, tc, pools, dram, n_kv):
    NKT = n_kv // 128
    big, ps, ps2, ptp, osp = pools
    xT_d, w_d, bias_d, outT_d = dram
    qchunks = _chunks(S)
    kchunks = _chunks(n_kv)
    assert all(wd == 512 for _, wd in qchunks)
    Exp = mybir.ActivationFunctionType.Exp

    # preload the exp table while DMAs run
    warm = big.tile([128, 1], F32, tag="warm", name="warm")
    nc.gpsimd.memset(warm[:], 1.0)
    nc.scalar.activation(warm[:], warm[:], Exp)

    # dummy matmuls on a memset tile: keep PE busy through the DMA-gated
    # head so HAM reaches full clock before the real work arrives
    dummy = big.tile([128, 512], BF16, tag="dummy", name="dummy")
    nc.gpsimd.memset(dummy[:], 1.0)
    dps = ps.tile([128, 1024], F32, tag="wide", name="dps")
    for _ in range(N_DUMMY):
        nc.tensor.matmul(
            dps[:, 0:512], dummy[:, 0:128], dummy[:], start=True, stop=True
        )

    w_s = big.tile([128, 6, NFI, 128], F16, tag="w", name="w_s")
    w_ap = w_d.ap()
    bias_s = big.tile([128, NKT], F32, tag="bias", name="bias_s")
    xT_ap = xT_d.ap().rearrange("(a p) t -> p a t", p=128)
    xts = big.tile([128, NFI, S], F16, tag="xts", name="xts")

    # DMA order = first-needed-first; chunk 0 stripped per-fi so the first
    # accumulations pipeline with arrival
    nc.sync.dma_start(w_s[:, 0], w_ap[:, 0])  # k0
    for fi in range(NFI):
        nc.sync.dma_start(xts[:, fi, 0:512], xT_ap[:, fi, 0:512])
    nc.sync.dma_start(bias_s[:], bias_d.ap())
    nc.sync.dma_start(w_s[:, 1], w_ap[:, 1])  # q0
    nc.sync.dma_start(w_s[:, 2], w_ap[:, 2])  # v0
    nc.sync.dma_start(w_s[:, 3], w_ap[:, 3])  # v1
    nc.sync.dma_start(xts[:, :, 512:1024], xT_ap[:, :, 512:1024])
    nc.sync.dma_start(w_s[:, 4], w_ap[:, 4])  # k1
    nc.sync.dma_start(w_s[:, 5], w_ap[:, 5])  # q1
    for off, wd in qchunks[2:]:
        nc.sync.dma_start(xts[:, :, off : off + wd], xT_ap[:, :, off : off + wd])
    xkv = xts  # kept keys are the first n_kv (host-permuted) columns

    qT = [big.tile([128, S], F16, tag=f"qT{p}", name=f"qT{p}") for p in range(2)]
    kT = [
        big.tile([128, n_kv], F16, tag=f"kT{p}", name=f"kT{p}") for p in range(2)
    ]
    va = big.tile([128, NKT, 4, 65], BF16, tag="va", name="va")
    ones = big.tile([128, 4, 1], F32, tag="ones", name="ones")
    nc.gpsimd.memset(ones[:], 1.0)
    for jt in range(NKT):
        nc.vector.tensor_copy(va[:, jt, :, 64:65], ones[:])

    def emit_k_chunk(p, off, wd):
        acc = ps.tile([128, 1024], F32, tag="wide", name="acc_k")
        for sub in range(0, wd, 512):
            sw = min(512, wd - sub)
            for fi in range(NFI):
                nc.tensor.matmul(
                    acc[:, sub : sub + sw],
                    w_s[:, PIECE_K[p], fi, :],
                    xkv[:, fi, off + sub : off + sub + sw],
                    start=(fi == 0),
                    stop=(fi == NFI - 1),
                )
        nc.vector.tensor_copy(kT[p][:, off : off + wd], acc[:, :wd])

    def emit_v_range(j0, j1):
        for jt in range(j0, j1):
            acc = ps.tile([128, 1024], F32, tag="wide", name="acc_v")
            for half in range(2):
                for fi in range(NFI):
                    nc.tensor.matmul(
                        acc[:, 128 * half : 128 * half + 128],
                        xkv[:, fi, jt * 128 : (jt + 1) * 128],
                        w_s[:, PIECE_V[half], fi, :],
                        start=(fi == 0),
                        stop=(fi == NFI - 1),
                    )
                nc.vector.tensor_copy(
                    va[:, jt, 2 * half : 2 * half + 2, 0:64],
                    acc[:, 128 * half : 128 * half + 128].rearrange(
                        "p (h d) -> p h d", h=2
                    ),
                )

    def emit_q(p, ci):
        off, wd = qchunks[ci]
        acc = ps.tile([128, 1024], F32, tag="wide", name="acc_q")
        for fi in range(NFI):
            nc.tensor.matmul(
                acc[:, :wd],
                w_s[:, PIECE_Q[p], fi, :],
                xts[:, fi, off : off + wd],
                start=(fi == 0),
                stop=(fi == NFI - 1),
            )
        nc.vector.tensor_copy(qT[p][:, off : off + wd], acc[:, :wd])

    # emission ordered by DMA arrival: work gated on x chunk i comes before
    # anything gated on chunk i+1; v tiles and later k chunks fold into
    # block 0's slot loop right before their first consumer
    emit_k_chunk(0, *kchunks[0])
    emit_q(0, 0)

    inner0 = {}
    for off, wd in kchunks[1:]:
        inner0.setdefault(off // 128, []).append(
            lambda off=off, wd=wd: emit_k_chunk(0, off, wd)
        )
    for jt in range(NKT):
        inner0.setdefault(min(jt + 1, NKT - 1), []).append(
            lambda jt=jt: emit_v_range(jt, jt + 1)
        )

    # fine-grained filler pieces: (needed_by_block, callable); one matmul each
    queue = []

    def q_pieces(p, ci, needed_by):
        off, wd = qchunks[ci]
        cell = []

        def mk(fi):
            def f():
                if not cell:
                    cell.append(
                        ps.tile([128, 1024], F32, tag="wide", name="acc_qf")
                    )
                nc.tensor.matmul(
                    cell[0][:, :wd],
                    w_s[:, PIECE_Q[p], fi, :],
                    xts[:, fi, off : off + wd],
                    start=(fi == 0),
                    stop=(fi == NFI - 1),
                )

            return f

        for fi in range(NFI):
            queue.append((needed_by, mk(fi)))
        queue.append(
            (
                needed_by,
                lambda: nc.vector.tensor_copy(
                    qT[p][:, off : off + wd], cell[0][:, :wd]
                ),
            )
        )

    def k_pieces(p, off, wd, needed_by):
        cell = []

        def mk(sub, sw, fi):
            def f():
                if not cell:
                    cell.append(
                        ps.tile([128, 1024], F32, tag="wide", name="acc_kf")
                    )
                nc.tensor.matmul(
                    cell[0][:, sub : sub + sw],
                    w_s[:, PIECE_K[p], fi, :],
                    xkv[:, fi, off + sub : off + sub + sw],
                    start=(fi == 0),
                    stop=(fi == NFI - 1),
                )

            return f

        for sub in range(0, wd, 512):
            sw = min(512, wd - sub)
            for fi in range(NFI):
                queue.append((needed_by, mk(sub, sw, fi)))
        queue.append(
            (
                needed_by,
                lambda: nc.vector.tensor_copy(
                    kT[p][:, off : off + wd], cell[0][:, :wd]
                ),
            )
        )

    nq = len(qchunks)
    q_pieces(0, 1, needed_by=1)
    q_pieces(0, 2, needed_by=2)
    q_pieces(0, 3, needed_by=3)
    for off, wd in kchunks:
        k_pieces(1, off, wd, needed_by=nq)
    for ci in range(nq):
        q_pieces(1, ci, needed_by=nq + ci)

    # ---- attention: linearized (pair, q-chunk, kv-block) stream ----
    blocks = [(p, ci) for p in range(2) for ci in range(len(qchunks))]
    T = len(blocks) * NKT

    def drain_required(bi):
        while queue and queue[0][0] <= bi:
            queue.pop(0)[1]()

    def pull(n):
        while n > 0 and queue:
            queue.pop(0)[1]()
            n -= 1

    def scores_exp(bi, j):
        p, ci = blocks[bi]
        coff, cw = qchunks[ci]
        sc = ps.tile([128, 1024], F32, tag="wide", name="sc")
        for i in range(2):
            lo = 64 * i
            nc.tensor.matmul(
                sc[:, 512 * i : 512 * i + cw],
                kT[p][lo : lo + 64, j * 128 : (j + 1) * 128],
                qT[p][lo : lo + 64, coff : coff + cw],
                start=True,
                stop=True,
            )
        pt = ptp.tile([128, 1024], BF16, tag="pt", name="pt")
        nc.scalar.activation(pt[:], sc[:], Exp, bias=bias_s[:, j : j + 1])
        return pt

    pv_cell = [None]

    def pv_mm(bi, j, pt):
        p, ci = blocks[bi]
        coff, cw = qchunks[ci]
        if j == 0:
            pv_cell[0] = ps2.tile([65, 1024], F32, tag="pv", name="pv")
        pv = pv_cell[0]
        for i in range(2):
            nc.tensor.matmul(
                pv[:, 512 * i : 512 * i + cw],
                va[:, j, 2 * p + i, :],
                pt[:, 512 * i : 512 * i + cw],
                start=(j == 0),
                stop=(j == NKT - 1),
            )
            if j == NKT - 1:
                o = osp.tile([65, 512], F32, tag="o", name="o")
                nc.vector.tensor_copy(o[:, :cw], pv[:, 512 * i : 512 * i + cw])
                lh = 2 * p + i
                nc.sync.dma_start(
                    outT_d.ap()[65 * lh : 65 * lh + 65, coff : coff + cw],
                    o[:, :cw],
                )

    # paced filler drain: meet needed_by deadlines (~2/slot up to midpoint),
    # then stretch the remainder across the rest of the stream so the PE
    # stays fed while ACT catches up on exps
    pend = []
    carry = 0.0
    for t in range(T):
        bi, j = divmod(t, NKT)
        if j == 0:
            drain_required(bi)
        if bi == 0:
            for f in inner0.get(j, []):
                f()
        pt = scores_exp(bi, j)
        pend.append((bi, j, pt))
        if t >= PULL_START_T and queue:
            if t < T // 2:
                pull(2)
            else:
                carry += len(queue) / max(1, T - 1 - t)
                n = int(carry)
                carry -= n
                pull(min(n, 3))
        if len(pend) > DEPTH:
            pv_mm(*pend.pop(0))
    while pend:
        pv_mm(*pend.pop(0))
    drain_required(len(blocks))


def _build(n_kv: int, reps: int = 1):
    """Build the per-core Bass graph. Same graph runs SPMD on all 8 cores."""
    nc = bacc.Bacc("TRN2", target_bir_lowering=False, debug=False)

    NKT = n_kv // 128
    xT_d = nc.dram_tensor("xT", [DIM, S], F16, kind="ExternalInput")
    w_d = nc.dram_tensor("w", [128, 6, NFI, 128], F16, kind="ExternalInput")
    bias_d = nc.dram_tensor("bias", [128, NKT], F32, kind="ExternalInput")
    outT_d = nc.dram_tensor("outT", [260, S], F32, kind="ExternalOutput")
    dram = (xT_d, w_d, bias_d, outT_d)

    with tile.TileContext(nc) as tc:
        with (
            tc.tile_pool(name="big", bufs=1) as big,
            tc.tile_pool(name="ps", bufs=3, space="PSUM") as ps,
            tc.tile_pool(name="ps2", bufs=1, space="PSUM") as ps2,
            tc.tile_pool(name="ptp", bufs=6) as ptp,
            tc.tile_pool(name="osp", bufs=4) as osp,
        ):
            pools = (big, ps, ps2, ptp, osp)
            for rep in range(reps):
                if rep:
                    tc.strict_bb_all_engine_barrier()
                _emit_body(nc, tc, pools, dram, n_kv)

    nc.compile()
    return nc


def _get_graph(n_kv: int, reps: int = 1):
    key = (n_kv, reps)
    if key not in _CACHE:
        _CACHE[key] = _build(n_kv, reps)
    return _CACHE[key]


def prepare(x, W_qkv, mask, reps: int = 1):
    """Host-side prep: returns (nc, in_maps, perms)."""
    x = np.asarray(x, dtype=np.float32)
    W_qkv = np.asarray(W_qkv, dtype=np.float32)
    mask = np.asarray(mask)

    keep = [np.nonzero(mask[b] != 0)[0] for b in range(B)]
    n_keep = max(len(k) for k in keep)
    n_kv = min(S, max(128, -(-n_keep // 128) * 128))

    # permute tokens: kept (unmasked) first, rest after; k/v use first n_kv
    perms, xT, biases = [], [], []
    for b in range(B):
        unkept = np.nonzero(mask[b] == 0)[0]
        perm = np.concatenate([keep[b], unkept])
        perms.append(perm)
        xT.append(np.ascontiguousarray(x[b][perm].T.astype(np.float16)))
        bv = np.full(n_kv, NEG, np.float32)
        bv[: len(keep[b])] = 0.0
        biases.append(np.ascontiguousarray(bv.reshape(-1, 128).T))

    wg = []
    for g in range(4):
        base = np.empty((128, 6, NFI, 128), np.float16)
        for pi, (c0, c1) in enumerate(W_PIECES):
            cols = np.empty((DIM, c1 - c0), np.float32)
            qkv_kind = [1, 0, 2, 2, 1, 0][pi]  # k0,q0,v0,v1,k1,q1 -> q/k/v base
            src0 = qkv_kind * DIM + 256 * g + (c0 % 256)
            cols[:] = W_qkv[:, src0 : src0 + (c1 - c0)]
            # [DIM, 128] -> [128 partitions, NFI, 128]
            base[:, pi] = (
                cols.reshape(NFI, 128, 128).transpose(1, 0, 2).astype(np.float16)
            )
        wg.append(base)

    in_maps = []
    for c in range(8):
        b, g = c // 4, c % 4
        in_maps.append({"xT": xT[b], "w": wg[g], "bias": biases[b]})

    nc = _get_graph(n_kv, reps)
    return nc, in_maps, perms


def assemble(results, perms):
    out = np.empty((B, S, DIM), np.float32)
    for c in range(8):
        b, g = c // 4, c % 4
        outT = results[c]["outT"]
        for i in range(4):
            h = 4 * g + i
            rows = outT[65 * i : 65 * i + 64]
            sums = outT[65 * i + 64]
            out[b, perms[b], 64 * h : 64 * (h + 1)] = (rows / sums).T
    return out


def run(x, W_qkv, mask, trace=False, tmpdir=None):
    nc, in_maps, perms = prepare(x, W_qkv, mask)
    res = run_bass_kernel_spmd(
        nc, in_maps, core_ids=list(range(8)), trace=trace, tmpdir=tmpdir
    )
    return assemble(res.results, perms), res


def kernel(x, W_qkv, mask):
    out, _ = run(x, W_qkv, mask)
    return out
